# revision 4
# baseline (speedup 1.0000x reference)
"""BitLinear (BitNet b1.58) TP kernel for 8 NeuronCores — v2 (PE-transpose).

y = fake_quant_act(x) @ ternary_absmean(W).T + bias

Sharding: W (and bias) split along out_features across 8 cores; x replicated.
Per core: W shard [1024, 8192] f32, x [16, 8192] f32 -> y shard [16, 1024].

Math (per core, exact reformulation of the reference):
  M      = max(|x|) over full x           (replicated -> identical per core)
  s      = 127 / M ;  xi = round(x*s)     (integers in [-127,127], exact bf16)
  alpha  = max(mean_k |W[o,:]|, eps)      (per out row)
  t      = clip(round(W/alpha), -1, 1)    (ternary)
  y[b,o] = (alpha[o]*M/127) * sum_k xi[b,k]*t[o,k] + bias[o]

v2 pipeline: the v1 kernel moved the ternary through a DMA xbar transpose
(16.8 MB/iter SBUF->SBUF on the same HWDGE ring as the 33.5 MB/iter W
loads -> ring-serialized ~250 us). v2 transposes on the PE instead:

  DMA (sync ring): W block halves [128, 4096] f32           (~101 us/iter)
  ACT: Abs pass with accum_out -> per-row |W| sums           (~58 us)
  DVE: q192 = bf16(w * (1/alpha) + 192)    one pass, f32->bf16, 2x mode
       (bf16 RNE in [128,256) has ulp=1 -> rounds to integer "magically")
  PE : is_transpose matmuls q192 tiles -> PSUM in *bf16*     (512 x 128c)
  DVE: PSUM->SBUF drain with fused ternary clip
       gbuf = max(min(q192T, 193), 191) = 192 + t   (bf16 2x_1p)
  PE : y^T[o,b] = sum_k gbuf[k,o] * xiT[k,b]  (64 k-tile matmuls/block,
       accumulating the *exact* integer sum_k (t+192)*xi in PSUM f32)
  DVE epilogue per block [128,16]: subtract the exact offset correction
       192*srow[b] (srow = sum_k xi), scale by alpha*M/127, add bias.

All integer paths (xi, t+192, srow, psum) are exact in f32/bf16, so the
only deviation from the reference is f32 summation-order noise in alpha
and threshold ties in the rounding magic.
"""

import contextlib
import os
import numpy as np

import concourse.bass as bass
import concourse.bacc as bacc
import concourse.mybir as mybir
from concourse import tile
from concourse.bass_utils import run_bass_kernel_spmd

F32 = mybir.dt.float32
BF16 = mybir.dt.bfloat16

B = 16          # batch
K = 8192        # in_features
OUT = 8192      # out_features
NCORES = 8
OSH = OUT // NCORES      # 1024 out rows per core
P = 128
NBLK = OSH // P          # 8 o-blocks per core
HALF = K // 2            # 4096 column half
KT = K // P              # 64 k-tiles
EPS = 1e-8
QMAX = 127.0
MAGIC32 = 1.5 * 2.0**23  # f32 round-to-nearest-even magic
OFF = 192.0              # bf16 round magic (ulp=1 in [128,256))

_CACHE = {}


def _build_bass(loop_r=None):
    nc = bacc.Bacc()
    x_d = nc.declare_dram_parameter("x", [B, K], F32, isOutput=False)
    w_d = nc.declare_dram_parameter("w", [OSH, K], F32, isOutput=False)
    bb_d = nc.declare_dram_parameter("biasb", [P, NBLK], F32, isOutput=False)
    id_d = nc.declare_dram_parameter("ident", [P, P], F32, isOutput=False)
    idb_d = nc.declare_dram_parameter("identb", [P, P], BF16, isOutput=False)
    y_d = nc.declare_dram_parameter("y", [OSH, B], F32, isOutput=True)

    with tile.TileContext(nc) as tc:
        with (
            tc.tile_pool(name="const", bufs=1) as cpool,
            tc.tile_pool(name="xp", bufs=2) as xpool,
            tc.tile_pool(name="wp", bufs=4) as wpool,
            tc.tile_pool(name="qp", bufs=3) as qpool,
            tc.tile_pool(name="gp", bufs=2) as gpool,
            tc.tile_pool(name="sm", bufs=2) as smpool,
            tc.tile_pool(name="al", bufs=4) as apool,
            tc.tile_pool(name="ps_t", bufs=2, space="PSUM") as ps_t,
            tc.tile_pool(name="ps_y", bufs=2, space="PSUM") as ps_y,
            tc.tile_pool(name="ps_r", bufs=1, space="PSUM") as ps_r,
            tc.tile_pool(name="ps_b", bufs=1, space="PSUM") as ps_b,
            tc.tile_pool(name="ps_c", bufs=1, space="PSUM") as ps_c,
        ):
            # ---- constants / persistent tiles ----
            ident = cpool.tile([P, P], F32, tag="ident")
            nc.sync.dma_start(ident[:], id_d[:])
            identb = cpool.tile([P, P], BF16, tag="identb")
            nc.sync.dma_start(identb[:], idb_d[:])
            ones_row = cpool.tile([1, P], F32, tag="ones_row")
            nc.vector.memset(ones_row[:], 1.0)
            biasb = cpool.tile([P, NBLK], F32, tag="biasb")
            nc.sync.dma_start(biasb[:], bb_d[:])

            loop = tc.For_i(0, loop_r) if loop_r else contextlib.nullcontext()
            with loop:
                # ---- x prep ----
                # load x as [128, 1024]: partition = (b*8+g), free = f,
                # global k = g*1024 + f
                x128 = xpool.tile([P, K // 8], F32, tag="x128")
                nc.scalar.dma_start(
                    x128[:], x_d[:].rearrange("b (g f) -> (b g) f", g=8)
                )
                mx128 = smpool.tile([P, 1], F32, tag="mx128")
                nc.vector.tensor_reduce(
                    mx128[:], x128[:], axis=mybir.AxisListType.X,
                    op=mybir.AluOpType.max, apply_absolute_value=True,
                )
                # cross-partition max via PE transpose then reduce
                trow_ps = ps_r.tile([1, P], F32, tag="trow")
                nc.tensor.transpose(trow_ps[:], mx128[:], ident[:])
                mrow = smpool.tile([1, P], F32, tag="mrow_sb")
                nc.vector.tensor_copy(mrow[:], trow_ps[:])
                mx = smpool.tile([1, 1], F32, tag="mx")
                nc.vector.tensor_reduce(
                    mx[:], mrow[:], axis=mybir.AxisListType.X,
                    op=mybir.AluOpType.max,
                )
                nc.vector.tensor_scalar(
                    mx[:], mx[:], EPS, None, op0=mybir.AluOpType.max
                )
                # inv_s = M/127 (epilogue scale); s = 127/M via recip+Newton
                inv_s = smpool.tile([1, 1], F32, tag="inv_s")
                nc.vector.tensor_scalar(
                    inv_s[:], mx[:], 1.0 / QMAX, None, op0=mybir.AluOpType.mult
                )
                r0 = smpool.tile([1, 1], F32, tag="r0")
                nc.vector.reciprocal(r0[:], mx[:])
                e1 = smpool.tile([1, 1], F32, tag="e1")
                nc.vector.tensor_tensor(e1[:], mx[:], r0[:], op=mybir.AluOpType.mult)
                nc.vector.tensor_scalar(
                    e1[:], e1[:], -1.0, 2.0,
                    op0=mybir.AluOpType.mult, op1=mybir.AluOpType.add,
                )
                rm = smpool.tile([1, 1], F32, tag="rm")
                nc.vector.tensor_tensor(rm[:], r0[:], e1[:], op=mybir.AluOpType.mult)
                s11 = smpool.tile([1, 1], F32, tag="s11")
                nc.vector.tensor_scalar(
                    s11[:], rm[:], QMAX, None, op0=mybir.AluOpType.mult
                )
                # broadcast (s, inv_s) -> [128, 2] via one K=1 matmul
                row2 = smpool.tile([1, 2], F32, tag="row2")
                nc.vector.tensor_copy(row2[:, 0:1], s11[:])
                nc.vector.tensor_copy(row2[:, 1:2], inv_s[:])
                bc_ps = ps_b.tile([P, 2], F32, tag="bcast")
                nc.tensor.matmul(bc_ps[:], ones_row[:], row2[:], start=True, stop=True)
                bc = smpool.tile([P, 2], F32, tag="bc_sb")
                nc.vector.tensor_copy(bc[:], bc_ps[:])

                # xi = round(x*s) via f32 magic; keep f32 copy for srow
                xr128 = xpool.tile([P, K // 8], F32, tag="xr128")
                nc.vector.tensor_scalar(
                    xr128[:], x128[:], bc[:, 0:1], MAGIC32,
                    op0=mybir.AluOpType.mult, op1=mybir.AluOpType.add,
                )
                xi_f = xpool.tile([P, K // 8], F32, tag="xi_f")
                nc.vector.tensor_scalar(
                    xi_f[:], xr128[:], MAGIC32, None,
                    op0=mybir.AluOpType.subtract,
                )
                xi_nat = xpool.tile([P, K // 8], BF16, tag="xi_nat")
                nc.vector.tensor_scalar(
                    xi_nat[:], xr128[:], MAGIC32, None,
                    op0=mybir.AluOpType.subtract,
                )
                # xbar: [bg, f1*128+f0] -> xi_t[f0, f1*128 + b*8 + g]
                xi_t = xpool.tile([P, KT * B], BF16, tag="xi_t")
                nc.scalar.dma_start(
                    xi_t[:].rearrange("p (f1 bg) -> p f1 bg", f1=8),
                    xi_nat[:],
                    transpose=True,
                )
                # lhsT view for k-tile kt=(g*8+f1): [128, 16], b strided by 8
                xi_v = xi_t[:].rearrange("p (f1 b g) -> p f1 g b", f1=8, b=B, g=8)

                # srow[b] = sum_k xi[b,k]: reduce xi_f rows -> [128,1] (=bg),
                # PE-transpose -> [1,128], reduce over g -> [1,16],
                # broadcast down partitions -> sc16[o,b] = 192*srow[b]
                sbg = smpool.tile([P, 1], F32, tag="sbg")
                nc.vector.tensor_reduce(
                    sbg[:], xi_f[:], axis=mybir.AxisListType.X,
                    op=mybir.AluOpType.add,
                )
                srow_ps = ps_r.tile([1, P], F32, tag="trow")
                nc.tensor.transpose(srow_ps[:], sbg[:], ident[:])
                srow128 = smpool.tile([1, P], F32, tag="srow128")
                nc.vector.tensor_copy(srow128[:], srow_ps[:])
                srow16 = smpool.tile([1, B], F32, tag="srow16")
                nc.vector.tensor_reduce(
                    srow16[:],
                    srow128[:].rearrange("r (b g) -> r b g", g=8),
                    axis=mybir.AxisListType.X,
                    op=mybir.AluOpType.add,
                )
                srow192 = smpool.tile([1, B], F32, tag="srow192")
                nc.vector.tensor_scalar(
                    srow192[:], srow16[:], OFF, None, op0=mybir.AluOpType.mult
                )
                sc_ps = ps_c.tile([P, B], F32, tag="sc16")
                nc.tensor.matmul(
                    sc_ps[:], ones_row[:], srow192[:], start=True, stop=True
                )
                sc16 = smpool.tile([P, B], F32, tag="sc16_sb")
                nc.vector.tensor_copy(sc16[:], sc_ps[:])

                # ---- main pipeline over o-blocks ----
                def emit_load(i):
                    wbs = [None, None]
                    asum = [None, None]
                    for h in range(2):
                        wb = wpool.tile([P, HALF], F32, tag="wb",
                                        name=f"wb{h}")
                        wbs[h] = wb
                        nc.sync.dma_start(
                            wb[:],
                            w_d[i * P:(i + 1) * P, h * HALF:(h + 1) * HALF],
                        )
                        scr = qpool.tile([P, HALF], BF16, tag="scr",
                                         name=f"scr{h}")
                        asum[h] = apool.tile([P, 1], F32, tag=f"asum{h}",
                                             name=f"asum{h}")
                        nc.scalar.activation(
                            scr[:], wb[:], mybir.ActivationFunctionType.Abs,
                            bias=0.0, scale=1.0 / K, accum_out=asum[h][:],
                        )
                    return wbs, asum

                def emit_process(i, wbs, asum):
                    alpha = apool.tile([P, 1], F32, tag="alpha")
                    nc.vector.tensor_tensor(
                        alpha[:], asum[0][:], asum[1][:],
                        op=mybir.AluOpType.add,
                    )
                    nc.vector.tensor_scalar(
                        alpha[:], alpha[:], EPS, None,
                        op0=mybir.AluOpType.max,
                    )
                    ra = apool.tile([P, 1], F32, tag="ra")
                    nc.vector.reciprocal(ra[:], alpha[:])
                    ea = apool.tile([P, 1], F32, tag="ea")
                    nc.vector.tensor_tensor(
                        ea[:], alpha[:], ra[:], op=mybir.AluOpType.mult
                    )
                    nc.vector.tensor_scalar(
                        ea[:], ea[:], -1.0, 2.0,
                        op0=mybir.AluOpType.mult, op1=mybir.AluOpType.add,
                    )
                    rinv = apool.tile([P, 1], F32, tag="rinv")
                    nc.vector.tensor_tensor(
                        rinv[:], ra[:], ea[:], op=mybir.AluOpType.mult
                    )
                    # alpha_s = alpha * M/127 (per-partition epilogue scale)
                    alpha_s = apool.tile([P, 1], F32, tag="alpha_s")
                    nc.vector.tensor_tensor(
                        alpha_s[:], alpha[:], bc[:, 1:2],
                        op=mybir.AluOpType.mult,
                    )

                    # q192 = bf16(w * rinv + 192): bf16 RNE rounds to int
                    qs = [None, None]
                    for h in range(2):
                        q = qpool.tile([P, HALF], BF16, tag="q192",
                                       name=f"q{h}")
                        nc.vector.tensor_scalar(
                            q[:], wbs[h][:], rinv[:], OFF,
                            op0=mybir.AluOpType.mult, op1=mybir.AluOpType.add,
                        )
                        qs[h] = q

                    # PE-transpose 8 chunks of 8 k-tiles; DVE drains each
                    # PSUM chunk to gbuf with the ternary clip fused
                    gbuf = gpool.tile([P, KT * P], BF16, tag="gbuf")
                    for j in range(8):
                        pst = ps_t.tile([P, 8 * P], BF16, tag="pst")
                        for c in range(8):
                            kt = j * 8 + c
                            h, kl = kt // 32, kt % 32
                            nc.tensor.transpose(
                                pst[:, c * P:(c + 1) * P],
                                qs[h][:, kl * P:(kl + 1) * P],
                                identb[:],
                            )
                        nc.vector.tensor_scalar(
                            gbuf[:, j * 8 * P:(j + 1) * 8 * P], pst[:],
                            OFF + 1.0, OFF - 1.0,
                            op0=mybir.AluOpType.min, op1=mybir.AluOpType.max,
                        )

                    # y^T[o,b] += sum_k gbuf[k,o] * xiT[k,b] (exact ints)
                    psg = ps_y.tile([P, B], F32, tag="psg")
                    for kt in range(KT):
                        g_, f1 = kt // 8, kt % 8
                        nc.tensor.matmul(
                            psg[:],
                            gbuf[:, kt * P:(kt + 1) * P],
                            xi_v[:, f1, g_, :],
                            start=(kt == 0),
                            stop=(kt == KT - 1),
                        )
                    # epilogue: y^T = (psg - 192*srow)*alpha_s + bias
                    ysb = smpool.tile([P, B], F32, tag="ysb")
                    nc.vector.tensor_tensor(
                        ysb[:], psg[:], sc16[:], op=mybir.AluOpType.subtract
                    )
                    nc.vector.tensor_scalar(
                        ysb[:], ysb[:], alpha_s[:], biasb[:, i:i + 1],
                        op0=mybir.AluOpType.mult, op1=mybir.AluOpType.add,
                    )
                    nc.scalar.dma_start(y_d[i * P:(i + 1) * P, :], ysb[:])

                pending = None
                for i in range(NBLK):
                    loaded = emit_load(i)
                    if pending is not None:
                        emit_process(i - 1, *pending)
                    pending = loaded
                emit_process(NBLK - 1, *pending)

    nc.finalize()
    return nc


def _get_nc():
    if "nc" not in _CACHE:
        _CACHE["nc"] = _build_bass()
    return _CACHE["nc"]


def _make_in_maps(x, weight, bias):
    import ml_dtypes

    x = np.ascontiguousarray(x, dtype=np.float32)
    weight = np.ascontiguousarray(weight, dtype=np.float32)
    bias = np.ascontiguousarray(bias, dtype=np.float32)
    ident = np.eye(P, dtype=np.float32)
    identb = np.eye(P, dtype=ml_dtypes.bfloat16)

    in_maps = []
    for c in range(NCORES):
        wsh = np.ascontiguousarray(weight[c * OSH:(c + 1) * OSH])
        bsh = np.ascontiguousarray(
            bias[c * OSH:(c + 1) * OSH].reshape(NBLK, P).T
        )
        in_maps.append(
            {"x": x, "w": wsh, "biasb": bsh, "ident": ident, "identb": identb}
        )
    return in_maps


def kernel(x, weight, bias):
    in_maps = _make_in_maps(x, weight, bias)
    nc = _get_nc()
    # The axon RL image lacks the NTFF profile hook; force the no-trace path.
    os.environ["BASS_NEVER_TRACE"] = "1"
    res = run_bass_kernel_spmd(nc, in_maps, list(range(NCORES)))
    _CACHE["last"] = res
    y = np.concatenate(
        [np.asarray(res.results[c]["y"]).T for c in range(NCORES)], axis=1
    )
    return np.ascontiguousarray(y, dtype=np.float32)


if __name__ == "__main__":
    rng = np.random.default_rng(0)
    x = rng.standard_normal((B, K), dtype=np.float32)
    w = rng.standard_normal((OUT, K), dtype=np.float32) * 0.01
    b = rng.standard_normal(OUT, dtype=np.float32) * 0.01
    y = kernel(x=x, weight=w, bias=b)
    print(y.shape, y.dtype)
